# revision 28
# baseline (speedup 1.0000x reference)
"""Trainium2 Bass kernel for the DifferentiableMemory scatter_memory problem.

Data-parallel over 8 NeuronCores: batch B=32768 is sharded into 8 x 4096 rows.
Host side does layout + light elementwise epilogue (rsqrt/sigmoid on [B]-sized
vectors), mirroring the host-side centroid pre-normalization; all O(B*D) math
(encoder MLP, cosine sims, top-k, importance net) runs on device.

Device dataflow (per core, 8 superblocks of 512 batch columns):
  activations live transposed [feature, batch]:
    x8        [128, 6, 512] fp8e4m3 = 8 * cue.T chunks  (fp8 halves both the
               HBM traffic and, via DoubleRow matmul, the layer-1 PE cycles)
    h1T       = gelu((W1*256).T @ x8 / 2048 + b1)  -> [256, 512] bf16
               (3 DoubleRow matmuls contract 2x128 K-rows per pass)
    encT      = W2.T @ h1T + b2             -> [128, 512] bf16
    ssq[b]    = ones.T @ (encT^2)           -> per-batch ||enc||^2 via PE
    sims[b,n] = encT.T @ centT_scaled       -> [128, 500] fp32 (centT
                 pre-divided by ||c||)
    top8      = nc.vector.max (one DVE instruction, sorted desc)
    icT       = gelu(imp1) @ iw2            (imp1 shares the DoubleRow sweep)
  raw top8 / ssq / ic ship to HBM; host applies rinv = rsqrt(ssq) to the top
  values (positive per-row scale preserves the top-k order; the reference's
  eps clamp never binds since ||enc||*||c|| >> 1e-8) and
  sigmoid(ic + ib2) * mean(emo) for the importance column.
"""

import os

import numpy as np
import ml_dtypes

BF16 = ml_dtypes.bfloat16
FP8 = ml_dtypes.float8_e4m3   # TRN FP8_EXP4 bit-compatible (max +-240, same inf/nan)

N_CORES = 8
B = 32768
BL = B // N_CORES          # 4096 rows per core
SB = 512                   # superblock: batch columns per iteration
NSB = BL // SB             # 8 superblocks
Q = SB // 128              # 4 x 128-row tiles per superblock
D = 768
H1 = 256
E = 128
N = 500
K = 5
WIDTHS = [512, 512, 512, 512, 512, 512, 512, 384, 128]
LQ = WIDTHS[-1] // 128     # tiles in the last block
XAT = 32 - LQ              # tiles shipped early
TOT = 902
DCH = D // 128             # 6 K-chunks
NDC = DCH // 2             # 3 DoubleRow K-pair chunks
S_X = 8.0                  # fp8 scale on activations
S_W = 256.0                # fp8 scale on weights
S_INV = 1.0 / (S_X * S_W)  # descale applied inside the gelu activation

_CACHE = {}


def _build_nc(has_ist, b2_zero):
    """Build the device kernel. has_ist: include the internal_state chunk
    (False when it is all-zeros, making its contribution exactly zero).
    b2_zero: skip the enc bias add (ACT copy instead of DVE add)."""
    import concourse.bacc as bacc
    import concourse.bass as bass
    import concourse.tile as tile
    from concourse import mybir

    f32 = mybir.dt.float32
    bf16 = mybir.dt.bfloat16
    fp8 = mybir.dt.float8e4
    AF = mybir.ActivationFunctionType
    PM = mybir.MatmulPerfMode
    ts = bass.ts

    nc = bacc.Bacc(None, target_bir_lowering=False,
               enable_asserts=False, enable_partition_id=False)

    x8 = nc.dram_tensor("x8", [128, DCH, BL], fp8, kind="ExternalInput")
    tailT = nc.dram_tensor("tailT", [6, BL], bf16, kind="ExternalInput")
    if has_ist:
        istT = nc.dram_tensor("istT", [E, BL], bf16, kind="ExternalInput")
    w1a = nc.dram_tensor("w1a", [128, NDC, 2, 128], fp8, kind="ExternalInput")
    w1h = nc.dram_tensor("w1h", [128, NDC, 2, 128], fp8, kind="ExternalInput")
    w1i = nc.dram_tensor("w1i", [128, NDC, 2, 64], fp8, kind="ExternalInput")
    w2 = nc.dram_tensor("w2", [128, 2, E], bf16, kind="ExternalInput")
    iw1 = nc.dram_tensor("iw1", [128, 2 if has_ist else 1, 64], bf16,
                         kind="ExternalInput")
    iw2 = nc.dram_tensor("iw2", [64, 1], bf16, kind="ExternalInput")
    b1 = nc.dram_tensor("b1", [128, 2], f32, kind="ExternalInput")
    b2 = nc.dram_tensor("b2", [128, 1], f32, kind="ExternalInput")
    ib1 = nc.dram_tensor("ib1", [64, 1], f32, kind="ExternalInput")
    centT = nc.dram_tensor("centT", [128, N], bf16, kind="ExternalInput")
    XT = NSB * Q               # 32 tiles of 128 rows
    XA = XAT                   # tiles finalized before the last block
    top8a = nc.dram_tensor("top8a", [128, XA * 8], f32, kind="ExternalOutput")
    top8b = nc.dram_tensor("top8b", [128, LQ * 8], f32, kind="ExternalOutput")
    sc = nc.dram_tensor("sc", [128, XT], f32, kind="ExternalOutput")
    encd = nc.dram_tensor("encd", [128, BL], bf16, kind="ExternalOutput")

    with tile.TileContext(nc) as tc:
        with (
            tc.tile_pool(name="const", bufs=1) as cpool,
            tc.tile_pool(name="work", bufs=3) as wpool,
            tc.tile_pool(name="acc", bufs=1) as apool,
            tc.tile_pool(name="psA", bufs=4, space="PSUM") as psA,
            tc.tile_pool(name="psS", bufs=4, space="PSUM") as psS,
        ):
            # PE warm-up gated only on a fast DVE memset. The HAM activity
            # monitor counts ARRAY duty-cycle, not instruction occupancy:
            # only wide (N=512, FWL-overlapped) matmuls read as "busy", so
            # the burst must be N=512 — a small-N stream never un-throttles
            # the 1.2GHz cold clock. 10 cold N=512 matmuls (~4.3us) flip the
            # clock at ~11us, just as the first x-block's DMA lands.
            scr = cpool.tile([128, 512], bf16)
            nc.vector.memset(scr[:], 0.0)
            ps_warm = psS.tile([128, 512], f32, tag="sims")
            for _ in range(10):
                nc.tensor.matmul(ps_warm[:], lhsT=scr[:, 0:128], rhs=scr[:],
                                 start=True, stop=True)

            # ---- consts. Two HWDGE queues issue in parallel, but early DMA
            # bandwidth (~130 GB/s aggregate, shared) is the scarce resource:
            # bytes are ordered by first use. scalar: w1 h0-chunk -> b1 ->
            # h1-chunk -> imp-chunk -> w2 -> centT; sync: x-stream + tiny
            # imp consts. GpSimd issues nothing at all — SWDGE use would add
            # its expensive dge_drain to the teardown. ----
            w1at = cpool.tile([128, NDC, 2, 128], fp8)
            w1ht = cpool.tile([128, NDC, 2, 128], fp8)
            w1it = cpool.tile([128, NDC, 2, 64], fp8)
            b1t = cpool.tile([128, 2], f32)
            nc.scalar.dma_start(w1at[:], w1a[:])
            nc.gpsimd.dma_start(b1t[:], b1[:])
            nc.scalar.dma_start(w1ht[:], w1h[:])
            nc.scalar.dma_start(w1it[:], w1i[:])
            w2t = cpool.tile([128, 2, E], bf16)
            iw1t = cpool.tile([128, 2 if has_ist else 1, 64], bf16)
            iw2t = cpool.tile([64, 1], bf16)
            ib1t = cpool.tile([64, 1], f32)
            centTt = cpool.tile([128, N], bf16)
            nc.scalar.dma_start(w2t[:], w2[:])
            nc.scalar.dma_start(centTt[:], centT[:])
            nc.gpsimd.dma_start(iw1t[:], iw1[:])
            nc.gpsimd.dma_start(ib1t[:], ib1[:])
            nc.gpsimd.dma_start(iw2t[:], iw2[:])
            if not b2_zero:
                b2t = cpool.tile([128, 1], f32)
                nc.gpsimd.dma_start(b2t[:], b2[:])

            # accumulators (raw outputs; the epilogue math runs on host)
            top8a_t = apool.tile([128, XA, 8], f32)
            top8b_t = apool.tile([128, LQ, 8], f32)
            sc_t = apool.tile([128, XT], f32)

            enc_pend = []
            dr_ok = not os.environ.get("KERNEL_NO_DR")
            # The tapered final blocks (384, 128) shrink the end-of-kernel
            # serial max8 drain on DVE to a single tile.
            widths = WIDTHS
            NB = len(widths)
            c0 = 0
            for bi, W in enumerate(widths):
                QB = W // 128
                xb = c0 // 128          # first 128-row tile index of block
                last = bi == NB - 1
                use_dr = dr_ok

                xt = wpool.tile([128, DCH, W], fp8, tag="xt")
                nc.sync.dma_start(xt[:], x8[:, :, c0 : c0 + W])
                xtail = wpool.tile([6, W], bf16, tag="xtail")
                nc.gpsimd.dma_start(xtail[:], tailT[:, c0 : c0 + W])
                if has_ist:
                    xti = wpool.tile([128, W], bf16, tag="xti")
                    nc.sync.dma_start(xti[:], istT[:, c0 : c0 + W])

                def xpair(c):
                    return xt[:, 2 * c : 2 * c + 2, :]

                # ---- fused layer 1: [W1 | imp_w1_cue].T @ xT as DoubleRow
                # K-pair sweeps; M-chunks h0, h1 -> h1 halves, imp -> head ----
                if use_dr:
                    # PE pipeline flush at plain->DoubleRow mode boundaries:
                    # the PE queue pulls LDWEIGHTS ahead of in-flight matmuls,
                    # and a perf-mode switch under that reorder wedges the
                    # exec unit (NRT_EXEC_UNIT_UNRECOVERABLE without this).
                    nc.tensor.drain()
                h1 = wpool.tile([128, 2, W], bf16, tag="h1")
                ps_imp = psA.tile([64, W], f32, tag="mm")
                for half in range(2):
                    ps = psA.tile([128, W], f32, tag="mm")
                    w1half = w1at if half == 0 else w1ht
                    for c in range(NDC):
                        if use_dr:
                            nc.tensor.matmul(
                                ps[:],
                                lhsT=w1half[:, c, :, :],
                                rhs=xpair(c),
                                start=(c == 0),
                                stop=(c == NDC - 1),
                                perf_mode=PM.DoubleRow,
                            )
                        else:
                            for s in range(2):
                                nc.tensor.matmul(
                                    ps[:],
                                    lhsT=w1half[:, c, s, :],
                                    rhs=xpair(c)[:, s, :],
                                    start=(c == 0 and s == 0),
                                    stop=(c == NDC - 1 and s == 1),
                                )
                    nc.scalar.activation(
                        h1[:, half, :], ps[:], AF.Gelu,
                        bias=b1t[:, half : half + 1], scale=S_INV,
                    )
                for c in range(NDC):
                    if use_dr:
                        nc.tensor.matmul(
                            ps_imp[:], lhsT=w1it[:, c, :, :], rhs=xpair(c),
                            start=(c == 0), stop=False, perf_mode=PM.DoubleRow,
                        )
                    else:
                        for s in range(2):
                            nc.tensor.matmul(
                                ps_imp[:], lhsT=w1it[:, c, s, :],
                                rhs=xpair(c)[:, s, :],
                                start=(c == 0 and s == 0), stop=False,
                            )

                if use_dr:
                    nc.tensor.drain()  # DoubleRow->plain boundary (see above)

                # ---- encoder layer 2: encT = W2.T @ h1T + b2 ----
                ps_enc = psA.tile([128, W], f32, tag="mm")
                for c in range(2):
                    nc.tensor.matmul(
                        ps_enc[:],
                        lhsT=w2t[:, c, :],
                        rhs=h1[:, c, :],
                        start=(c == 0),
                        stop=(c == 1),
                    )
                encb = wpool.tile([128, W], bf16, tag="encb", bufs=4)
                if b2_zero:
                    nc.scalar.activation(encb[:], ps_enc[:], AF.Copy)
                else:
                    nc.vector.tensor_scalar_add(encb[:], ps_enc[:], b2t[:])
                if last:
                    # early ship on the now-idle scalar HWDGE queue: tiles of
                    # all earlier blocks are final; overlaps the last block's
                    # matmuls, no head-of-line block of the sync x-stream.
                    nc.scalar.dma_start(top8a[:], top8a_t[:])
                # ---- importance net layer 1: istate + tail chunks ----
                if has_ist:
                    nc.tensor.matmul(
                        ps_imp[:], lhsT=iw1t[:, 1, :], rhs=xti[:],
                        start=False, stop=False,
                    )
                nc.tensor.matmul(
                    ps_imp[:], lhsT=iw1t[0:6, 0, :], rhs=xtail[:],
                    start=False, stop=True,
                )
                himp = wpool.tile([64, W], bf16, tag="himp")
                nc.scalar.activation(himp[:], ps_imp[:], AF.Gelu,
                                     bias=ib1t[:], scale=S_INV)

                # ---- sims + top8 per 128-row tile (max8 reads PSUM) ----
                for q in range(QB):
                    ps_sims = psS.tile([128, N], f32, tag="sims")
                    nc.tensor.matmul(
                        ps_sims[:],
                        lhsT=encb[:, ts(q, 128)],
                        rhs=centTt[:],
                        start=True,
                        stop=True,
                    )
                    dst = (top8b_t[:, q, :] if last
                           else top8a_t[:, xb + q, :])
                    nc.vector.max(dst, ps_sims[:])

                # ---- importance scalar head ----
                ps_ic = psS.tile([128, QB], f32, tag="sims")
                for q in range(QB):
                    nc.tensor.matmul(
                        ps_ic[:, q : q + 1],
                        lhsT=himp[:, ts(q, 128)],
                        rhs=iw2t[:],
                        start=True,
                        stop=True,
                    )
                nc.scalar.activation(sc_t[:, xb : xb + QB], ps_ic[:],
                                     AF.Copy)

                # ship encb for the host-side ||enc||^2 — TWO blocks delayed:
                # the sync queue is FIFO, so an encd issue that still waits on
                # its encb would head-of-line-block the x-stream prefetch
                # behind it (measured as a 1.7us PE gap with a 1-block delay).
                enc_pend.append((c0, W, encb))
                if len(enc_pend) > 2:
                    pc0, pW, pencb = enc_pend.pop(0)
                    nc.scalar.dma_start(encd[:, pc0 : pc0 + pW], pencb[:])
                if last:
                    for pc0, pW, pencb in enc_pend:
                        nc.scalar.dma_start(encd[:, pc0 : pc0 + pW], pencb[:])
                c0 += W

            # ---- tail: ship the last block's raw tiles. sc's wait (the
            # last ic copy) clears before top8b's (the last max8), so this
            # FIFO order adds nothing after the final max8 but one issue. ----
            nc.sync.dma_start(sc[:], sc_t[:])
            nc.sync.dma_start(top8b[:], top8b_t[:])

    nc.compile()
    return nc


def _prep_inputs(has_ist, cue, internal_state, reward, timestamp,
                 emotional_state, centroids, enc_w1, imp_w1):
    f32 = np.float32

    tail = np.empty((6, B), dtype=f32)
    tail[0] = reward[:, 0]
    tail[1] = timestamp[:, 0]
    tail[2:6] = emotional_state.T
    tail_bf = tail.astype(BF16)
    cue8 = np.clip(cue * S_X, -240.0, 240.0).astype(FP8)
    ist_bf = internal_state.astype(BF16) if has_ist else None

    w1e = np.concatenate([enc_w1, imp_w1[:D]], axis=1) * S_W     # [768, 320]
    w1q = np.clip(w1e, -240.0, 240.0).astype(FP8)
    # [768, 320] -> [NDC, 2, 128, M] -> [128(k), NDC, 2, M]
    w1p = w1q.reshape(NDC, 2, 128, H1 + 64).transpose(2, 0, 1, 3)
    w1a = np.ascontiguousarray(w1p[:, :, :, 0:128])
    w1h = np.ascontiguousarray(w1p[:, :, :, 128:256])
    w1i = np.ascontiguousarray(w1p[:, :, :, 256:320])

    nchi = 2 if has_ist else 1
    iw1p = np.zeros((nchi * 128, 64), dtype=f32)
    iw1p[0:6] = imp_w1[TOT - 6 : TOT] * (S_X * S_W)  # tail chunk, scale-matched
    if has_ist:
        iw1p[128 : 128 + E] = imp_w1[D : D + E] * (S_X * S_W)
    iw1 = np.ascontiguousarray(
        iw1p.astype(BF16).reshape(nchi, 128, 64).transpose(1, 0, 2)
    )

    in_maps = []
    for i in range(N_CORES):
        sl = slice(i * BL, (i + 1) * BL)
        m = dict(w1a=w1a, w1h=w1h, w1i=w1i, iw1=iw1)
        # x8[p, c, b] = 8*cue[b, 128c+p]
        m["x8"] = np.ascontiguousarray(
            cue8[sl].T.reshape(DCH, 128, BL).transpose(1, 0, 2)
        )
        m["tailT"] = np.ascontiguousarray(tail_bf[:, sl])
        if has_ist:
            m["istT"] = np.ascontiguousarray(ist_bf[sl].T)
        in_maps.append(m)
    return in_maps


def kernel(cue, internal_state, reward, timestamp, emotional_state, centroids,
           enc_w1, enc_b1, enc_w2, enc_b2, imp_w1, imp_b1, imp_w2, imp_b2,
           top_k, **run_kwargs):
    assert int(top_k) == K, f"kernel hardcodes top_k={K}, got {top_k}"
    from concourse.bass_utils import run_bass_kernel_spmd

    f32 = np.float32
    cue = np.asarray(cue, f32)
    internal_state = np.asarray(internal_state, f32)
    reward = np.asarray(reward, f32)
    timestamp = np.asarray(timestamp, f32)
    emotional_state = np.asarray(emotional_state, f32)
    centroids = np.asarray(centroids, f32)
    enc_w1 = np.asarray(enc_w1, f32)
    enc_b1 = np.asarray(enc_b1, f32)
    enc_w2 = np.asarray(enc_w2, f32)
    enc_b2 = np.asarray(enc_b2, f32)
    imp_w1 = np.asarray(imp_w1, f32)
    imp_b1 = np.asarray(imp_b1, f32)
    imp_w2 = np.asarray(imp_w2, f32)
    imp_b2 = np.asarray(imp_b2, f32)

    has_ist = bool(np.any(internal_state))
    b2_zero = not np.any(enc_b2)
    key = ("nc", has_ist, b2_zero)
    if key not in _CACHE:
        _CACHE[key] = _build_nc(has_ist, b2_zero)
    nc = _CACHE[key]

    in_maps = _prep_inputs(has_ist, cue, internal_state, reward, timestamp,
                           emotional_state, centroids, enc_w1, imp_w1)
    shared = dict(
        w2=np.ascontiguousarray(
            enc_w2.astype(BF16).reshape(2, 128, E).transpose(1, 0, 2)),
        iw2=np.ascontiguousarray(imp_w2.astype(BF16).reshape(64, 1)),
        b1=np.ascontiguousarray(enc_b1.reshape(2, 128).T),
        b2=np.ascontiguousarray(enc_b2.reshape(128, 1)),
        ib1=np.ascontiguousarray(imp_b1.reshape(64, 1)),
        centT=np.ascontiguousarray(
            (centroids / np.linalg.norm(centroids, axis=1)[:, None]).T
        ).astype(BF16),
    )
    for m in in_maps:
        m.update(shared)

    res = run_bass_kernel_spmd(
        nc, in_maps, core_ids=list(range(N_CORES)), **run_kwargs
    )

    # host epilogue: rinv scaling of the top values, sigmoid for importance
    ib2_s = float(imp_b2.reshape(-1)[0])
    emo_mean = emotional_state.mean(axis=-1)                     # [B]
    out = np.empty((B, K + 1), dtype=f32)
    for i in range(N_CORES):
        r = res.results[i]
        # [128, X, 8] -> batch rows x*128+p
        t8 = np.concatenate(
            [r["top8a"].reshape(128, XAT, 8),
             r["top8b"].reshape(128, LQ, 8)], axis=1)
        rows = slice(i * BL, (i + 1) * BL)
        top5 = t8[:, :, 0:K].transpose(1, 0, 2).reshape(BL, K)
        enc = r["encd"].astype(f32)                  # [128 feat, BL]
        ssq = np.einsum("fb,fb->b", enc, enc)
        ic = r["sc"].reshape(128, NSB * Q).T.reshape(BL)
        rinv = 1.0 / np.sqrt(ssq)
        out[rows, 0:K] = top5 * rinv[:, None]
        out[rows, K] = (1.0 / (1.0 + np.exp(-(ic + ib2_s)))) * emo_mean[rows]
    _CACHE["last_results"] = res
    return out
